# revision 8
# baseline (speedup 1.0000x reference)
"""Trainium2 Bass kernel for the raw-reshape RoPE attention problem.

Math structure (verified against the reference):
  The reference reshapes [B, N, H*D] -> [B, H, N, D] with a *raw* reshape
  (no transpose).  Viewing the projection [2048, 1024] as [32768, 64],
  head h covers rows [h*2048, (h+1)*2048) -- i.e. head h only sees input
  tokens [h*128, (h+1)*128), and the final output rows [h*128, (h+1)*128)
  depend only on head h.  So the 32 (b, h) pairs are fully independent:
  4 pairs per NeuronCore, no collectives.

  Within a head we use the row permutation j' = s*128 + t (orig j = t*16+s,
  s = weight block, t = token).  Softmax/attention are invariant to a
  consistent row permutation of q/k/v; it makes every on-device layout
  change a contiguous [64, 128] block copy.

  RoPE rotates pair (2i, 2i+1) of each 64-channel block by the angle of
  position j-1 (row j=0 unrotated).  We pre-permute wq/wk rows on the host
  so even channels land in [s*64, s*64+32) and odd in [s*64+32, s*64+64),
  making the on-device rotation pure contiguous-block arithmetic.  The
  permutation cancels inside the q.k dot products.

Per (b, h) pair on device (all matmuls bf16, fp32 accumulation):
  Qp/Kp/Vp = Xp @ Wt           (lhsT = x.T blocks, host-pretransposed)
  rope(Qp), rope(Kp) on DVE    -> bf16 [128 tok, 1024 chan]
  PE-transpose -> k2t packs key-chunk PAIRS into the two 64-row halves of
  the array; q2t holds the 64 head dims DUPLICATED in both halves.
  Scores run ROW-TILED (64x128 array packing, tile_position (0,0)/(64,0)):
  two key chunks' scoresT compute concurrently on the two array halves,
  halving scores PE time (contraction dim is only 64).
  E = exp(s/8) on ScalarE per chunk; AV in full 128x128 mode with
  vsb = [ones(64) | V(64)] so acc rows 0:64 hold the softmax sums
  REPLICATED 64x -- normalization needs no cross-partition broadcast:
  recip on DVE directly, then two strided muls assemble out_tok.T.
  final = out_tok @ wo.T (deferred into the next pair's attention window;
  the last pair's wo is split into two half-contractions so only the
  second half trails the final attention).

Startup: whole-tensor weight DMAs split in halves across the four issue
queues (wq/wk first, wv/wo behind) + ~3.5us of dummy PE warmup matmuls so
the HAM clock-gate reaches 2.4 GHz before the first projection.
"""

import numpy as np
import ml_dtypes

N_CORES = 8
B, N, DIM = 2, 2048, 1024
H, HD = 16, 64
PAIRS_PER_CORE = 4
BF16 = ml_dtypes.bfloat16

_CACHE = {}


def _rope_tables():
    """cos/sin tables [128 t, 16 s, 32 i]; (t=0, s=0) is the unrotated row."""
    inv = 1.0 / (10000.0 ** (np.arange(0, HD, 2, dtype=np.float32) / HD))
    pos = np.arange(128 * 16, dtype=np.float32).reshape(128, 16) - 1.0  # j-1
    ang = pos[:, :, None] * inv[None, None, :]          # [128, 16, 32]
    c = np.cos(ang).astype(np.float32)
    s = np.sin(ang).astype(np.float32)
    c[0, 0, :] = 1.0
    s[0, 0, :] = 0.0
    return c, s


def _chan_perm():
    """c = s*64 + 2i + par -> c' = s*64 + par*32 + i."""
    perm = np.zeros(DIM, np.int64)
    for s in range(16):
        i = np.arange(32)
        perm[s * 64 + i] = s * 64 + 2 * i
        perm[s * 64 + 32 + i] = s * 64 + 2 * i + 1
    return perm


def _build_nc():
    import concourse.mybir as mybir
    import concourse.tile as tile
    from concourse import bacc
    from concourse.masks import make_identity

    dt = mybir.dt
    AF = mybir.ActivationFunctionType

    nc = bacc.Bacc("TRN2", target_bir_lowering=False, debug=False,
                   num_devices=N_CORES)

    xt_d = nc.declare_dram_parameter("xt", [PAIRS_PER_CORE, 128, 8, 128],
                                     dt.bfloat16, isOutput=False)
    w_d = {}
    for name in ("wq", "wk", "wv", "wo"):
        w_d[name] = nc.declare_dram_parameter(name + "t", [128, 8, 1024],
                                              dt.bfloat16, isOutput=False)
    rc_d = nc.declare_dram_parameter("ropec", [128, 16, 32], dt.float32,
                                     isOutput=False)
    rs_d = nc.declare_dram_parameter("ropes", [128, 16, 32], dt.float32,
                                     isOutput=False)
    out_d = nc.declare_dram_parameter("out", [PAIRS_PER_CORE, 128, 1024],
                                      dt.float32, isOutput=True)


    with tile.TileContext(nc) as tc:
        with (
            tc.tile_pool(name="wts", bufs=1) as wts,
            tc.tile_pool(name="const", bufs=1) as constp,
            tc.tile_pool(name="xin", bufs=4) as xin,
            tc.tile_pool(name="tmp", bufs=4) as tmpp,
            tc.tile_pool(name="qkr", bufs=3) as qkrp,
            tc.tile_pool(name="vsb", bufs=2) as vsbp,
            tc.tile_pool(name="qkt", bufs=2) as qktp,
            tc.tile_pool(name="esb", bufs=8) as esbp,
            tc.tile_pool(name="rsb", bufs=2) as rsbp,
            tc.tile_pool(name="osb", bufs=2) as osbp,
            tc.tile_pool(name="psA", bufs=1, space="PSUM") as psA,
            tc.tile_pool(name="psS", bufs=2, space="PSUM") as psS,
            tc.tile_pool(name="psW", bufs=2, space="PSUM") as psW,
        ):
            # pair-0 activations + rope tables first (tiny), then the four
            # weight tensors as half-tensor DMAs spread over all four issue
            # queues: wq/wk stream first at full aggregate HBM bandwidth,
            # wv/wo queue behind them.
            xs0 = xin.tile([128, 8, 128], dt.bfloat16, tag="xt")
            nc.sync.dma_start(xs0[:], xt_d[0])
            rc = constp.tile([128, 16, 32], dt.float32, tag="rc")
            rs = constp.tile([128, 16, 32], dt.float32, tag="rs")
            nc.scalar.dma_start(rc[:], rc_d[:])
            nc.scalar.dma_start(rs[:], rs_d[:])
            # touch Exp immediately so ACT_TABLE_LOAD runs during DMA warmup
            warm = constp.tile([1, 1], dt.float32, tag="warm")
            nc.scalar.activation(warm[:], rc[0:1, 0:1, 0:1], AF.Exp)
            w_sb = {}
            for name in ("wq", "wk", "wv", "wo"):
                w_sb[name] = wts.tile([128, 8, 1024], dt.bfloat16, tag=name,
                                      name=name)
            # hardware-DGE queues only (gpsimd DMA is software-DGE: it
            # occupies the Pool engine for the whole transfer).  wq/wk
            # first (QK proj gates everything), wv right behind (first AV
            # needs it ~19us in), wo last.
            for eng, name, sh in (
                (nc.sync, "wq", slice(0, 4)), (nc.sync, "wq", slice(4, 8)),
                (nc.scalar, "wk", slice(0, 4)), (nc.scalar, "wk", slice(4, 8)),
                (nc.sync, "wv", slice(0, 4)), (nc.scalar, "wv", slice(4, 8)),
                (nc.sync, "wo", slice(0, 4)), (nc.scalar, "wo", slice(4, 8)),
            ):
                eng.dma_start(w_sb[name][:, sh, :], w_d[name][:, sh, :])

            ident = constp.tile([128, 128], dt.bfloat16, tag="id")
            make_identity(nc, ident[:])

            # HAM warmup that survives dead-code elimination: a 32-matmul
            # accumulation chain (ident @ ident summed 32x) whose result,
            # scaled by exactly 1/32, BECOMES the identity used by every
            # PE transpose.  ~3.5us of PE activity during the DMA wait
            # brings the clock gate to 2.4 GHz before the first projection.
            wmm = psW.tile([128, 128], dt.float32, tag="w", name="warmmm")
            for i in range(32):
                nc.tensor.matmul(wmm[:], ident[:], ident[:],
                                 start=(i == 0), stop=(i == 31))
            ident2 = constp.tile([128, 128], dt.bfloat16, tag="id2")
            nc.scalar.activation(ident2[:], wmm[:], AF.Copy, scale=1.0 / 32)

            fin_work = []   # deferred output projection of the previous pair

            def emit_fin(otok_t, p_idx):
                osb = osbp.tile([128, 1024], dt.float32, tag="osb")
                for nt in range(2):
                    fin = psW.tile([128, 512], dt.float32, tag="w")
                    for kk in range(8):
                        nc.tensor.matmul(
                            fin[:],
                            otok_t[:, kk, :],
                            w_sb["wo"][:, kk, nt * 512:(nt + 1) * 512],
                            start=(kk == 0), stop=(kk == 7))
                    nc.vector.tensor_copy(osb[:, nt * 512:(nt + 1) * 512],
                                          fin[:])
                nc.sync.dma_start(out_d[p_idx], osb[:])

            def proj_transp(p):
                """projections + rope + transposes for pair p; returns
                (q2t, k2t, vsb) ready for attention."""
                if p == 0:
                    xs = xs0
                else:
                    xs = xin.tile([128, 8, 128], dt.bfloat16, tag="xt",
                                  name=f"xs{p}")
                    nc.sync.dma_start(xs[:], xt_d[p])

                # q2t: head dims duplicated into both 64-row halves (each
                # scores row-tile streams its own SBUF partition range).
                # k2t: key-chunk PAIRS packed [lo half: chunk 2cp,
                # hi half: chunk 2cp+1].  vsb cols 0:64 all-ones -> AV
                # accumulates the softmax sums replicated on rows 0:64.
                q2t = qktp.tile([128, 16, 128], dt.bfloat16, tag="q2t",
                                name=f"q2t{p}")
                k2t = qktp.tile([128, 8, 128], dt.bfloat16, tag="k2t",
                                name=f"k2t{p}")
                vsb = vsbp.tile([128, 16, 128], dt.bfloat16, tag="v",
                                name=f"vsb{p}")
                nc.gpsimd.memset(vsb[:, :, 0:64], 1.0)

                qr = qkrp.tile([128, 16, 2, 32], dt.bfloat16, tag="qr",
                               name=f"qr{p}")
                kr = qkrp.tile([128, 16, 2, 32], dt.bfloat16, tag="kr",
                               name=f"kr{p}")

                ppool = psS if p == 0 else psW
                ptag = "s" if p == 0 else "w"

                def proj_qk(tname, dst):
                    for nt in range(2):
                        sh = slice(nt * 8, (nt + 1) * 8)
                        pp = ppool.tile([128, 8, 2, 32], dt.float32, tag=ptag)
                        for kk in range(8):
                            nc.tensor.matmul(
                                pp[:],
                                xs[:, kk, :],
                                w_sb[tname][:, kk, nt * 512:(nt + 1) * 512],
                                start=(kk == 0), stop=(kk == 7))
                        xe, xo = pp[:, :, 0, :], pp[:, :, 1, :]
                        t1 = tmpp.tile([128, 8, 32], dt.float32, tag="t1")
                        t2 = tmpp.tile([128, 8, 32], dt.float32, tag="t2")
                        nc.vector.tensor_mul(t1[:], xe, rc[:, sh, :])
                        nc.vector.tensor_mul(t2[:], xo, rs[:, sh, :])
                        nc.vector.tensor_sub(dst[:, sh, 0, :], t1[:], t2[:])
                        t3 = tmpp.tile([128, 8, 32], dt.float32, tag="t1")
                        t4 = tmpp.tile([128, 8, 32], dt.float32, tag="t2")
                        nc.vector.tensor_mul(t3[:], xe, rs[:, sh, :])
                        nc.vector.tensor_mul(t4[:], xo, rc[:, sh, :])
                        nc.vector.tensor_add(dst[:, sh, 1, :], t3[:], t4[:])

                proj_qk("wq", qr)
                proj_qk("wk", kr)

                # PE transposes; copy fan-out split DVE/Pool.  tp rows
                # 0:64 = s-block 2kk, rows 64:128 = s-block 2kk+1.
                # all copies on DVE and emitted lo/hi back-to-back so the
                # two q2t/k2t halves become ready together -- the Tile
                # scheduler otherwise splits the row-tiled score pairs
                # (asymmetric readiness), which serializes the concurrency
                for kk in range(8):
                    tpq = psW.tile([128, 128], dt.bfloat16, tag="w")
                    nc.tensor.transpose(
                        tpq[:], qr[:, 2 * kk:2 * kk + 2, :, :], ident2[:])
                    for sub in range(2):
                        s = 2 * kk + sub
                        nc.vector.tensor_copy(q2t[0:64, s, :],
                                              tpq[sub * 64:(sub + 1) * 64, :])
                        nc.vector.tensor_copy(q2t[64:128, s, :],
                                              tpq[sub * 64:(sub + 1) * 64, :])
                    tpk = psW.tile([128, 128], dt.bfloat16, tag="w")
                    nc.tensor.transpose(
                        tpk[:], kr[:, 2 * kk:2 * kk + 2, :, :], ident2[:])
                    nc.vector.tensor_copy(k2t[0:64, kk, :], tpk[0:64, :])
                    nc.vector.tensor_copy(k2t[64:128, kk, :], tpk[64:128, :])

                # V projection last: only needed once attention reaches AV.
                for nt in range(2):
                    sh = slice(nt * 8, (nt + 1) * 8)
                    pp = psW.tile([128, 8, 64], dt.float32, tag="w")
                    for kk in range(8):
                        nc.tensor.matmul(
                            pp[:],
                            xs[:, kk, :],
                            w_sb["wv"][:, kk, nt * 512:(nt + 1) * 512],
                            start=(kk == 0), stop=(kk == 7))
                    nc.vector.tensor_copy(vsb[:, sh, 64:128], pp[:])
                return q2t, k2t, vsb

            def att_half(p, jh, tiles, otok, mid=None, late=None):
                q2t, k2t, vsb = tiles
                acc = psA.tile([128, 8, 128], dt.float32, tag="acc")

                def av(e, c):
                    for jt in range(2):
                        nc.tensor.matmul(
                            acc[:, jt * 4:(jt + 1) * 4, :],
                            vsb[:, c, :],
                            e[:, jt * 512:(jt + 1) * 512],
                            start=(c == 0), stop=(c == 15),
                            skip_group_check=True)

                pend = []
                for cp in range(8):
                    # row-tiled scores: chunk 2cp on array rows 0:63,
                    # chunk 2cp+1 on rows 64:127, running concurrently
                    sctA = psS.tile([128, 1024], dt.float32, tag="s")
                    sctB = psS.tile([128, 1024], dt.float32, tag="s")
                    for jt in range(2):
                        s0 = jh * 8 + jt * 4
                        nc.tensor.matmul(
                            sctA[:, jt * 512:(jt + 1) * 512],
                            k2t[0:64, cp, :],
                            q2t[0:64, s0:s0 + 4, :],
                            start=True, stop=True, tile_position=(0, 0))
                        nc.tensor.matmul(
                            sctB[:, jt * 512:(jt + 1) * 512],
                            k2t[64:128, cp, :],
                            q2t[64:128, s0:s0 + 4, :],
                            start=True, stop=True, tile_position=(64, 0))
                    eA = esbp.tile([128, 1024], dt.bfloat16, tag="e")
                    eB = esbp.tile([128, 1024], dt.bfloat16, tag="e")
                    nc.scalar.activation(eA[:], sctA[:], AF.Exp, scale=0.125)
                    nc.scalar.activation(eB[:], sctB[:], AF.Exp, scale=0.125)
                    pend.append((eA, 2 * cp))
                    pend.append((eB, 2 * cp + 1))
                    # AV runs a couple of chunk-pairs behind so exp stays fed
                    while len(pend) > 4:
                        av(*pend.pop(0))
                    if cp == 2 and mid is not None:
                        mid()
                    if cp == 4 and late is not None:
                        late()
                while pend:
                    av(*pend.pop(0))

                # normalize: acc rows 0:64 hold the softmax sums replicated,
                # so the reciprocal broadcasts for free.  Two strided muls:
                # even s-blocks -> otok rows 0:64, odd -> rows 64:128.
                rsb = rsbp.tile([64, 8, 128], dt.float32, tag="r")
                nc.vector.reciprocal_approx_fast(out=rsb[:],
                                                 in_=acc[0:64, :, :])
                for par in range(2):
                    nc.vector.tensor_mul(
                        otok[par * 64:par * 64 + 64,
                             jh * 4:(jh + 1) * 4, :],
                        acc[64:128, par:8:2, :],
                        rsb[:, par:8:2, :])

            def emit_fin_part(otok_t, p_idx, half, state):
                """half-contraction of the wo projection (last pair only)."""
                if half == 0:
                    state["fins"] = [psW.tile([128, 512], dt.float32,
                                              tag="w", name=f"fin3_{nt}")
                                     for nt in range(2)]
                for nt in range(2):
                    fin = state["fins"][nt]
                    for kk in range(half * 4, half * 4 + 4):
                        nc.tensor.matmul(
                            fin[:],
                            otok_t[:, kk, :],
                            w_sb["wo"][:, kk, nt * 512:(nt + 1) * 512],
                            start=(kk == 0), stop=(kk == 7),
                            skip_group_check=True)
                if half == 1:
                    osb = osbp.tile([128, 1024], dt.float32, tag="osb")
                    for nt in range(2):
                        nc.vector.tensor_copy(
                            osb[:, nt * 512:(nt + 1) * 512],
                            state["fins"][nt][:])
                    nc.sync.dma_start(out_d[p_idx], osb[:])

            # pair-level software pipeline: proj/transp of pair p+1 is
            # emitted between pair p's two attention halves, fin of p-1
            # right after it -- both land in attention's ACT-bound PE slack
            tiles = proj_transp(0)
            nxt_box = [None]
            fin3 = {}
            for p in range(PAIRS_PER_CORE):
                otok = osbp.tile([128, 8, 128], dt.bfloat16, tag="otok",
                                 name=f"otok{p}")
                att_half(p, 0, tiles, otok)

                if p + 1 < PAIRS_PER_CORE:
                    def mid(p=p):
                        nxt_box[0] = proj_transp(p + 1)

                    def late():
                        while fin_work:
                            emit_fin(*fin_work.pop(0))
                else:
                    # last pair: drain pending fins early, then start the
                    # first half of our own wo so only the second half
                    # trails the final attention
                    def mid():
                        while fin_work:
                            emit_fin(*fin_work.pop(0))

                    def late(otok=otok, p=p):
                        emit_fin_part(otok, p, 0, fin3)

                att_half(p, 1, tiles, otok, mid=mid, late=late)
                if p + 1 < PAIRS_PER_CORE:
                    fin_work.append((otok, p))
                    tiles = nxt_box[0]
                else:
                    emit_fin_part(otok, p, 1, fin3)

            while fin_work:
                emit_fin(*fin_work.pop(0))

    nc.compile()
    return nc



def _get_nc():
    if "nc" not in _CACHE:
        _CACHE["nc"] = _build_nc()
    return _CACHE["nc"]


def _prep_inputs(x, wq, wk, wv, wo):
    perm = _chan_perm()
    ropec, ropes = _rope_tables()

    def wt(w):
        # [out_chan, dim] -> transposed, partition-major [128, 8, 1024]
        return np.ascontiguousarray(
            w.T.reshape(8, 128, DIM).transpose(1, 0, 2)).astype(BF16)

    wqt = wt(wq[perm, :])
    wkt = wt(wk[perm, :])
    wvt = wt(wv)
    wot = wt(wo)

    in_maps = []
    for core in range(N_CORES):
        xts = np.empty((PAIRS_PER_CORE, 128, 8, 128), BF16)
        for pl in range(PAIRS_PER_CORE):
            pg = core * PAIRS_PER_CORE + pl
            b, h = pg // H, pg % H
            X = x[b, h * 128:(h + 1) * 128, :]      # [128 tok, 1024]
            xts[pl] = np.ascontiguousarray(
                X.T.reshape(8, 128, 128).transpose(1, 0, 2)).astype(BF16)
        in_maps.append({
            "xt": xts,
            "wqt": wqt, "wkt": wkt, "wvt": wvt, "wot": wot,
            "ropec": ropec, "ropes": ropes,
        })
    return in_maps


def run_sharded(x, wq, wk, wv, wo, trace=False, **run_kwargs):
    """Build + run on 8 cores; returns (full_output, BassKernelResults)."""
    from concourse.bass_utils import run_bass_kernel_spmd

    nc = _get_nc()
    in_maps = _prep_inputs(np.asarray(x, np.float32), np.asarray(wq, np.float32),
                           np.asarray(wk, np.float32), np.asarray(wv, np.float32),
                           np.asarray(wo, np.float32))
    res = run_bass_kernel_spmd(nc, in_maps, core_ids=list(range(N_CORES)),
                               trace=trace, **run_kwargs)
    out = np.empty((B, N, DIM), np.float32)
    for core in range(N_CORES):
        o = np.asarray(res.results[core]["out"], np.float32)
        for pl in range(PAIRS_PER_CORE):
            pg = core * PAIRS_PER_CORE + pl
            b, h = pg // H, pg % H
            out[b, h * 128:(h + 1) * 128, :] = o[pl]
    return out, res


def kernel(x, wq, wk, wv, wo):
    out, _ = run_sharded(x, wq, wk, wv, wo, trace=False)
    return out
